# revision 5
# baseline (speedup 1.0000x reference)
"""Betti3D loss kernel for Trainium2 (8 NeuronCores, data-parallel over batch).

Reference computation (see problem):
    p_down  = trilinear_resize(p_hat, (32, 32, 8))   # [B, C, 32, 32, 8]
    conf[b] = max(p_down[b, struct_id])
    out     = sum((1 - conf) * betti_error) / B

With input [B, C, 160, 160, 64] -> (32, 32, 8) the resize scales are exactly
(5, 5, 8), so with torch/jax half-pixel centers the source coordinates are:
    D axis: 5*i + 2      (weight exactly 0 -> pure gather)
    H axis: 5*j + 2      (weight exactly 0 -> pure gather)
    W axis: 8*k + 3.5    (weight exactly 0.5 -> 0.5*(x[8k+3] + x[8k+4]))
Therefore
    p_down[b, c, i, j, k] = 0.5 * (x[b,c,5i+2,5j+2,8k+3] + x[b,c,5i+2,5j+2,8k+4])
and conf[b] = max_{i,j,k} 0.5*(a+b), computed exactly on device.

Per-core kernel (one batch sample per core), raw Bass (no TileContext):
  - The 1024 needed 256 B rows (256 KB) of channel struct_id are gathered by
    TWO dynamic DMAs in parallel: one on the Sync HWDGE queue (qSPDynamicHW)
    and one on the Activation HWDGE queue (qActDynamicHW).  A single queue's
    descriptor pipeline was the measured bottleneck (~1.8-3.4 ns/descriptor);
    two queues split the 1024 descriptors.
  - DVE: two fused TENSOR_TENSOR_REDUCE ops (one per DMA half so each waits
    on exactly one semaphore - this toolchain rejects >1 wait/instruction):
    out = (x[...,3::8] + x[...,4::8]) * 0.5, accum = max -> [128, 1].
  - GpSimd: partition_all_reduce(max) -> the scalar conf in every partition
    (variant "gs"), so the output DMA is a single 4 B descriptor instead of
    128 partition-strided ones.
  - Sync: 4 B output DMA, no completion semaphore attached (nothing waits on
    it; the NEFF-end runtime quiescence drains the queue).

betti_error is 1 only for struct_id == 2 ('Myo'); for the other structures the
loss is exactly 0 and no device work is needed.
"""

import os

import numpy as np

_TARGETS = ((1, 0, 0), (1, 0, 0), (1, 1, 0), (1, 0, 0))
_BETTI_FALLBACK = (1, 0, 0)

_N_CORES = 8
_IN_SHAPE = (4, 160, 160, 64)  # per-sample [C, D, H, W]

_module_cache: dict = {}
LAST_RESULTS = None  # BassKernelResults of the most recent device run


def _ensure_ntff_hook():
    """Make trace=True safe anywhere: the image's antenv package lacks
    axon_hooks, whose absence crashes run_bass_kernel_spmd's trace path.
    Install a shim module and register the ctypes NTFF hook when available
    (hook=None degrades to bass_utils' graceful 'skip trace' path)."""
    import sys
    import types

    if "antenv.axon_hooks" not in sys.modules:
        try:
            import antenv.axon_hooks  # noqa: F401
        except ImportError:
            mod = types.ModuleType("antenv.axon_hooks")
            mod._hook = None
            mod.set_axon_ntff_profile_hook = lambda h: setattr(mod, "_hook", h)
            mod.get_axon_ntff_profile_hook = lambda: mod._hook
            sys.modules["antenv.axon_hooks"] = mod
            try:
                from trn_agent_boot.trn_boot import _ntff_profile_via_ctypes

                hook = _ntff_profile_via_ctypes("/opt/axon/libaxon_pjrt.so")
                if hook is not None:
                    mod.set_axon_ntff_profile_hook(hook)
            except Exception:
                pass
    # No S3 in this container; keep NTFF artifacts local.
    from concourse import bass_utils

    if getattr(bass_utils.upload_artifacts, "__name__", "") != "<lambda>":
        bass_utils.upload_artifacts = lambda tmpdir: tmpdir


def _strip_const_memsets(m, idle):
    """Drop Bass.__init__ overhead this kernel doesn't need: the const-*
    memsets (they'd open the NTFF 'useful' window ~0.7 us early), the
    init all-engine barrier (Drain/EventSemaphore pairs — walrus's own
    starting CoreBarrier already aligns the engines), and register setup
    on engines that execute nothing."""
    for function in m.functions:
        for block in function.blocks:
            keep = []
            for inst in block.instructions:
                tn = type(inst).__name__
                eng = str(getattr(inst, "engine", "")).split(".")[-1]
                if tn in ("InstDrain", "InstEventSemaphore"):
                    continue
                if tn == "InstMemset" and inst.outs and getattr(
                        inst.outs[0], "memref", "").startswith("const-"):
                    continue
                if eng in idle and tn in ("InstRegisterMove", "InstNoOp"):
                    continue
                keep.append(inst)
            if len(keep) != len(block.instructions):
                block.instructions[:] = keep


def _merge_blocks(m):
    """No control flow: fold basic blocks chained by unconditional branches
    into one block and drop the chaining branches."""
    for fn in m.functions:
        blocks = list(fn.blocks)
        if len(blocks) <= 1:
            continue
        names = [b.name for b in blocks]
        merged = []
        for bi, b in enumerate(blocks):
            nxt = names[bi + 1] if bi + 1 < len(names) else None
            for inst in b.instructions:
                if (type(inst).__name__ == "InstUnconditionalBranch"
                        and getattr(inst, "target", None) == nxt):
                    continue
                merged.append(inst)
        blocks[0].instructions[:] = merged
        fn.blocks[:] = [blocks[0]]


def _build(struct_id: int, variant: str):
    import concourse.bass as bass
    import concourse.bass_isa as bass_isa
    from concourse import mybir

    nc = bass.Bass("TRN2", target_bir_lowering=False, debug=False,
                   num_devices=_N_CORES)
    x = nc.dram_tensor("x", list(_IN_SHAPE), mybir.dt.float32,
                       kind="ExternalInput").ap()
    out_len = 1 if variant == "gs" else 128
    o = nc.dram_tensor("o", [out_len], mybir.dt.float32,
                       kind="ExternalOutput").ap()

    t = nc.alloc_sbuf_tensor("t", [128, 512], mybir.dt.float32)
    scr = nc.alloc_sbuf_tensor("scr", [128, 64], mybir.dt.float32)
    red = nc.alloc_sbuf_tensor("red", [128, 1], mybir.dt.float32)

    sem_a = nc.alloc_semaphore("sem_a")
    sem_b = nc.alloc_semaphore("sem_b")
    sem_v = nc.alloc_semaphore("sem_v")

    # --- input gather: two HWDGE queues, 512 descriptors each -------------
    # d rows 2,7,...,157; first 16 to partitions 0-63 (sync queue), last 16
    # to partitions 64-127 (activation queue).  Partition p = d_idx*4 + j//8,
    # free = (j%8, w).
    nc.sync.dma_start(t[0:64, :], x[struct_id, 2:78:5, 2::5, :]).then_inc(
        sem_a, 16)
    nc.scalar.dma_start(t[64:128, :], x[struct_id, 82:158:5, 2::5, :]).then_inc(
        sem_b, 16)

    # --- (a+b) + max-reduce per DMA half (host applies the exact 0.5) ------
    # (TENSOR_TENSOR_REDUCE is rejected by this walrus build: "ISA wrong
    # length" in CoreV2GenImpl::visitInstISA.  Plain TT + TensorReduce.)
    tv = t[:].rearrange("p (j w) -> p j w", w=64)
    sv = scr[:].rearrange("p (j k) -> p j k", k=8)
    tt0 = nc.vector.tensor_tensor(
        out=sv[0:64], in0=tv[0:64, :, 3::8], in1=tv[0:64, :, 4::8],
        op=mybir.AluOpType.add)
    tt0._wait_ge(sem_a, 16)
    r0 = nc.vector.reduce_max(red[0:64, :], scr[0:64, :],
                              axis=mybir.AxisListType.X)
    r0.then_inc(sem_v, 1)
    tt1 = nc.vector.tensor_tensor(
        out=sv[64:128], in0=tv[64:128, :, 3::8], in1=tv[64:128, :, 4::8],
        op=mybir.AluOpType.add)
    tt1._wait_ge(sem_b, 16)
    r1 = nc.vector.reduce_max(red[64:128, :], scr[64:128, :],
                              axis=mybir.AxisListType.X)
    r1.then_inc(sem_v, 1)

    # walrus requires ≥1 sync update on a dynamic DMA (sync::Update front()
    # asserts non-empty), so the output DMA gets a sem nothing waits on.
    sem_o = nc.alloc_semaphore("sem_o")
    if variant == "gs":
        # --- cross-partition max on GpSimd, then a 1-descriptor output ----
        red2 = nc.alloc_sbuf_tensor("red2", [128, 1], mybir.dt.float32)
        sem_g = nc.alloc_semaphore("sem_g")
        ar = nc.gpsimd.partition_all_reduce(
            red2[:], red[:], channels=128, reduce_op=bass_isa.ReduceOp.max)
        ar._wait_ge(sem_v, 2)
        ar.then_inc(sem_g, 1)
        od = nc.sync.dma_start(o[0:1], red2[0:1, 0:1])
        od._wait_ge(sem_g, 1)
        od.then_inc(sem_o, 16)
        idle = {"PE"}
    else:  # "v128": DMA all 128 partition maxima; host finishes the max
        od = nc.sync.dma_start(o[:], red[:])
        od._wait_ge(sem_v, 2)
        od.then_inc(sem_o, 16)
        idle = {"PE", "Pool"}

    _strip_const_memsets(nc.m, idle)
    _merge_blocks(nc.m)
    return nc


def kernel(p_hat: np.ndarray, struct_id) -> np.ndarray:
    global LAST_RESULTS
    sid = int(struct_id)
    target = _TARGETS[sid]
    betti_error = sum(abs(_BETTI_FALLBACK[k] - target[k]) for k in range(3))
    B = p_hat.shape[0]
    if betti_error == 0:
        return np.zeros((), dtype=p_hat.dtype)

    from concourse import bass_utils

    assert B == _N_CORES and tuple(p_hat.shape[1:]) == _IN_SHAPE, (
        f"kernel hardcoded for shape (8, 4, 160, 160, 64), got {p_hat.shape}"
    )
    variant = os.environ.get("BETTI_KVARIANT", "gs")
    key = (sid, variant)
    if key not in _module_cache:
        _module_cache[key] = _build(sid, variant)
    nc = _module_cache[key]

    p_hat = np.ascontiguousarray(p_hat, dtype=np.float32)
    in_maps = [{"x": p_hat[b]} for b in range(B)]
    trace = bool(int(os.environ.get("BETTI_TRACE", "0")))
    if trace or os.environ.get("BASS_TRACE"):
        _ensure_ntff_hook()
    res = bass_utils.run_bass_kernel_spmd(
        nc, in_maps, core_ids=list(range(_N_CORES)), trace=trace
    )
    LAST_RESULTS = res

    per_core = np.stack([r["o"].reshape(-1) for r in res.results])  # [8, k]
    m = per_core.max(axis=1).astype(np.float32)  # device computed max(a+b)
    conf = np.float32(0.5) * m                   # exact power-of-2 scaling
    total = np.sum((np.float32(1.0) - conf) * np.float32(betti_error),
                   dtype=np.float32)
    out = total / np.float32(max(B, 1))
    return np.asarray(out, dtype=p_hat.dtype)
